# revision 21
# baseline (speedup 1.0000x reference)
"""Chamfer loss on 8 Trainium2 NeuronCores.

Data-parallel over batch B=8: one batch element per core. Per core the
[N, M] = [2048, 2048] squared-distance matrix is produced on the
TensorEngine as 64 matmuls (16 x-strips x 4 N=512 blocks) using the
expansion  d2[i,j] = |x|^2 + |y|^2 - 2 x.y  with 18 augmented bf16-split
contraction rows (prepared host-side, O(N) work).

Reduction plan (v3):
 - Row mins (X side): one VectorE tensor_scalar(min) with accum_out=min
   per strip over the bf16 SBUF copy -- runs in 4x DVE mode.
 - The PSUM->SBUF f32->bf16 drain runs on the ScalarEngine for most
   strips; for D_FUSED strips the drain is fused into the DVE
   tensor_scalar (drain + row-min accum in one 1x op) to balance the
   ACT/DVE load.
 - Col mins (Y side): strips (some pre-merged pairwise with one
   tensor_tensor(min)) are transposed by the DMA xbar engine
   (dma_start_transpose) into [128(y), NT, 128(x)] layout; one 4x
   tensor_scalar accum per 128-wide y-block then yields the col mins.
   No running col-min chain, no PE transposes, no TensorReduce.
 - Since sqrt is monotone, sqrt is applied to the 32 minima only;
   the ScalarEngine sqrt's accum_out produces the per-partition sum.
Device ships per-partition sums of sqrt(min); host finishes with a
128-element sum per core and the batch mean.
"""

import numpy as np

B, N, M, D = 8, 2048, 2048, 2
P = 128            # partition tile (X rows per strip)
TN = N // P        # 16 strips
NBLK = 512         # matmul moving free dim (one PSUM bank of fp32)
K_AUG = 18         # contraction rows: 6 hi/lo/lolo products per coord + split norms
BIG = 3.0e38

# tuning knobs
FUSED_STRIPS = (6, 10)  # strips whose drain+rowmin is fused on DVE
N_PAIRS = 2        # strip pairs merged with one TT(min) before transpose
                   # (first 2*N_PAIRS strips); rest are transposed directly

_nc_cache = {}
last_results = None
TRACE = False


def _build(reps=1):
    """reps>1 wraps the whole computation in a hardware For_i loop --
    used only for steady-state timing measurements."""
    import concourse.bacc as bacc
    import concourse.tile as tile
    from concourse import mybir
    from contextlib import nullcontext

    f32 = mybir.dt.float32
    bf16 = mybir.dt.bfloat16
    Alu = mybir.AluOpType

    NT = TN - N_PAIRS  # transpose units

    nc = bacc.Bacc(
        "TRN2",
        target_bir_lowering=False,
        debug=False,
        enable_asserts=False,
        num_devices=B,
    )
    lhs_d = nc.dram_tensor("lhs_aug", [K_AUG, N], bf16, kind="ExternalInput")
    rhs_d = nc.dram_tensor("rhs_aug", [K_AUG, M], bf16, kind="ExternalInput")
    out_d = nc.dram_tensor("out", [P, 1], f32, kind="ExternalOutput")

    with tile.TileContext(nc) as tc:
        with (
            tc.tile_pool(name="const", bufs=1) as const,
            tc.tile_pool(name="strips", bufs=6) as strips,
            tc.tile_pool(name="paccs", bufs=2) as paccs,
            tc.tile_pool(name="scratch", bufs=1) as scratch,
            tc.tile_pool(name="psum_d2", bufs=2, space="PSUM") as pd2,
        ):
            lhsT = const.tile([K_AUG, N], bf16)
            rhsT = const.tile([K_AUG, M], bf16)
            nc.sync.dma_start(out=lhsT, in_=lhs_d.ap())
            nc.sync.dma_start(out=rhsT, in_=rhs_d.ap())

            accT = const.tile([P, NT, TN, P], bf16)  # transposed strips
            xy = const.tile([P, 2 * TN], f32)   # [:, :TN] row mins, [:, TN:] col mins
            xyc2 = const.tile([P, TN], f32)     # col-min partials (mid units)
            xyc3 = const.tile([P, TN], f32)     # col-min partials (last unit)
            dist = const.tile([P, 2 * TN], f32)
            sums = const.tile([P, 1], f32)
            junk = scratch.tile([P, M], bf16)
            NT1 = NT // 2  # col-min stage-1 unit count (early transposes)
            junk2 = scratch.tile([P, NT, P], bf16)

            # preload the sqrt activation table during the ramp so the
            # ~2.7us ACT_TABLE_LOAD is not paid in the serial tail
            warm = const.tile([1, 1], f32)
            nc.vector.memset(warm, 1.0)
            nc.scalar.sqrt(warm, warm)

            loop_cm = tc.For_i(0, reps, 1) if reps > 1 else nullcontext()
            with loop_cm:
                pend = None     # first bstrip of an open pair
                unit = 0        # next transpose-unit slot in accT
                stage1 = False  # col-min stage 1 issued
                stage2a = False  # col-min stage 2a issued
                for s in range(TN):
                    d2 = pd2.tile([P, M], f32, name="d2")
                    for j in range(M // NBLK):
                        nc.tensor.matmul(
                            d2[:, j * NBLK : (j + 1) * NBLK],
                            lhsT[:, s * P : (s + 1) * P],
                            rhsT[:, j * NBLK : (j + 1) * NBLK],
                            start=True,
                            stop=True,
                        )
                    bstrip = strips.tile([P, M], bf16, name="bstrip")
                    if s in FUSED_STRIPS:
                        # drain + row-min fused in one DVE op (1x, f32 PSUM)
                        nc.vector.tensor_scalar(
                            bstrip, d2, BIG, None, op0=Alu.min, op1=Alu.min,
                            accum_out=xy[:, s : s + 1],
                        )
                    else:
                        nc.scalar.copy(bstrip, d2)
                        nc.vector.tensor_scalar(
                            junk, bstrip, BIG, None, op0=Alu.min, op1=Alu.min,
                            accum_out=xy[:, s : s + 1],
                        )
                    # col-min feed: pair-merge the first 2*N_PAIRS strips,
                    # transpose everything via the DMA xbar
                    if s < 2 * N_PAIRS:
                        if pend is None:
                            pend = bstrip
                        else:
                            pacc = paccs.tile([P, M], bf16, name="pacc")
                            nc.vector.tensor_tensor(pacc, pend, bstrip, op=Alu.min)
                            nc.sync.dma_start_transpose(
                                out=accT[:, unit, :, :], in_=pacc
                            )
                            pend = None
                            unit += 1
                    else:
                        # ACT-drained strips: issue the transpose from the
                        # ACT HWDGE queue (input was produced two ops earlier
                        # on the same queue -> no cross-engine wait); DVE-
                        # drained (fused) strips go via the SP queue.
                        eng = nc.sync if s in FUSED_STRIPS else nc.scalar
                        eng.dma_start_transpose(
                            out=accT[:, unit, :, :], in_=bstrip
                        )
                        unit += 1
                    # col-min stage 1 / 2a: reduce transpose units that
                    # already exist while later strips drain
                    if unit == NT1 and not stage1:
                        stage1 = True
                        for c in range(TN):
                            nc.vector.tensor_scalar(
                                junk2[:, :NT1, :], accT[:, :NT1, c, :], BIG, None,
                                op0=Alu.min, op1=Alu.min,
                                accum_out=xy[:, TN + c : TN + c + 1],
                            )
                    if unit == NT - 1 and not stage2a:
                        stage2a = True
                        for c in range(TN):
                            nc.vector.tensor_scalar(
                                junk2[:, NT1 : NT - 1, :],
                                accT[:, NT1 : NT - 1, c, :], BIG, None,
                                op0=Alu.min, op1=Alu.min,
                                accum_out=xyc2[:, c : c + 1],
                            )

                # col mins stage 2b: the last transpose unit, then merge all
                for c in range(TN):
                    nc.vector.tensor_scalar(
                        junk2[:, NT - 1, :], accT[:, NT - 1, c, :], BIG, None,
                        op0=Alu.min, op1=Alu.min,
                        accum_out=xyc3[:, c : c + 1],
                    )
                nc.vector.tensor_tensor(xyc2, xyc2, xyc3, op=Alu.min)
                nc.vector.tensor_tensor(
                    xy[:, TN : 2 * TN], xy[:, TN : 2 * TN], xyc2, op=Alu.min
                )
                # d2 minima can round slightly negative; clamp before sqrt
                nc.vector.tensor_scalar_max(xy, xy, 0.0)
                nc.scalar.activation(
                    dist, xy, mybir.ActivationFunctionType.Sqrt, accum_out=sums
                )
                nc.sync.dma_start(out=out_d.ap(), in_=sums)

    nc.compile()
    return nc


def _split3(v):
    """3-way bf16 split: v ~= h + l + ll with ~2^-27 relative residual."""
    import ml_dtypes

    bf = ml_dtypes.bfloat16
    h = v.astype(bf)
    r = v - h.astype(np.float32)
    l = r.astype(bf)
    ll = (r - l.astype(np.float32)).astype(bf)
    return h, l, ll


def _prep_core(x, y):
    """Host-side per-core operand prep: O(N) layout, norms, bf16 splits.

    Summing lhsT[k]*rhs[k] over the 18 rows reconstructs
    |x|^2 + |y|^2 - 2 x.y with ~2^-27-scale absolute error (products of
    bf16 values are exact in the fp32 PSUM accumulator; only the
    representation residual and the dropped l*ll cross terms remain).
    Per coordinate (w = -2y): h*h', h*l', l*h', l*l', h*ll', ll*h'.
    Norms enter as 3-way splits against ones.
    """
    import ml_dtypes

    bf = ml_dtypes.bfloat16
    x = np.ascontiguousarray(x, dtype=np.float32)
    y = np.ascontiguousarray(y, dtype=np.float32)
    w = -2.0 * y
    nx = (x.astype(np.float64) ** 2).sum(axis=1).astype(np.float32)
    ny = (y.astype(np.float64) ** 2).sum(axis=1).astype(np.float32)

    lhs = np.empty((K_AUG, N), dtype=bf)
    rhs = np.empty((K_AUG, M), dtype=bf)
    k = 0
    for c in range(2):
        xh, xl, xll = _split3(x[:, c])
        wh, wl, wll = _split3(w[:, c])
        for a, b in ((xh, wh), (xh, wl), (xl, wh), (xl, wl), (xh, wll), (xll, wh)):
            lhs[k], rhs[k] = a, b
            k += 1
    one_n = np.ones(N, bf)
    one_m = np.ones(M, bf)
    for part in _split3(nx):
        lhs[k], rhs[k] = part, one_m
        k += 1
    for part in _split3(ny):
        lhs[k], rhs[k] = one_n, part
        k += 1
    assert k == K_AUG
    return {"lhs_aug": lhs, "rhs_aug": rhs}


def run(pds, pred_pds, reps=1, trace=None):
    global last_results
    from concourse import bass_utils

    pds = np.asarray(pds)
    pred_pds = np.asarray(pred_pds)
    assert pds.shape == (B, N, D) and pred_pds.shape == (B, M, D)

    if reps not in _nc_cache:
        _nc_cache[reps] = _build(reps)
    nc = _nc_cache[reps]

    in_maps = [_prep_core(pds[b], pred_pds[b]) for b in range(B)]
    last_results = bass_utils.run_bass_kernel_spmd(
        nc, in_maps, core_ids=list(range(B)),
        trace=TRACE if trace is None else trace,
    )
    vals = [
        float(last_results.results[b]["out"].sum()) / (2.0 * N) for b in range(B)
    ]
    return np.float32(np.mean(vals))


def kernel(pds, pred_pds):
    return run(pds, pred_pds, reps=1)


# revision 25
# speedup vs baseline: 2.0777x; 2.0777x over previous
"""Chamfer loss on 8 Trainium2 NeuronCores.

Data-parallel over batch B=8: one batch element per core. Per core the
[N, M] = [2048, 2048] squared-distance matrix is produced on the
TensorEngine as 64 matmuls (16 x-strips x 4 N=512 blocks) using the
expansion  d2[i,j] = |x|^2 + |y|^2 - 2 x.y  with 18 augmented bf16-split
contraction rows (prepared host-side, O(N) work).

Reduction plan (v4):
 - Drain: one ScalarEngine copy per strip (PSUM f32 -> SBUF bf16,
   FD=2048); for FUSED_STRIPS the drain runs on the VectorEngine as a
   tensor_scalar(min, BIG) whose accum_out simultaneously yields the
   strip row-min (drain + row-min in one 1x op).
 - Row mins (X side): one VectorE tensor_scalar with accum_out=min per
   ACT-drained strip over the bf16 SBUF copy -- 4x DVE mode, 2048 elems
   in ~594 ns.
 - Col mins (Y side): running elementwise tensor_tensor(min) into a
   bf16 accumulator (2x DVE mode); partition direction finished with
   16 PE transposes and one multi-dim TensorReduce.
 - Since sqrt is monotone, sqrt is applied to the 2*16 minima only; the
   ScalarEngine sqrt's accum_out directly produces the per-partition
   sum of distances.
Device ships per-partition sums of sqrt(min); host finishes with a
128-element sum per core and the batch mean.
"""

import numpy as np

B, N, M, D = 8, 2048, 2048, 2
P = 128            # partition tile (X rows per strip)
TN = N // P        # 16 strips
NBLK = 512         # matmul moving free dim (one PSUM bank of fp32)
K_AUG = 18         # contraction rows: 6 hi/lo/lolo products per coord + split norms
BIG = 3.0e38

# strips whose drain+rowmin is fused on DVE (balances ACT vs DVE load)
FUSED_STRIPS = ()

_nc_cache = {}
last_results = None
TRACE = False


def _build(reps=1):
    """reps>1 wraps the whole computation in a hardware For_i loop --
    used only for steady-state timing measurements."""
    import concourse.bacc as bacc
    import concourse.tile as tile
    from concourse import mybir
    from concourse.masks import make_identity
    from contextlib import nullcontext

    f32 = mybir.dt.float32
    bf16 = mybir.dt.bfloat16
    Alu = mybir.AluOpType

    nc = bacc.Bacc(
        "TRN2",
        target_bir_lowering=False,
        debug=False,
        enable_asserts=False,
        num_devices=B,
    )
    lhs_d = nc.dram_tensor("lhs_aug", [K_AUG, N], bf16, kind="ExternalInput")
    rhs_d = nc.dram_tensor("rhs_aug", [K_AUG, M], bf16, kind="ExternalInput")
    out_d = nc.dram_tensor("out", [P, 1], f32, kind="ExternalOutput")

    with tile.TileContext(nc) as tc:
        with (
            tc.tile_pool(name="const", bufs=1) as const,
            tc.tile_pool(name="strips", bufs=4) as strips,
            tc.tile_pool(name="scratch", bufs=1) as scratch,
            tc.tile_pool(name="psum_d2", bufs=2, space="PSUM") as pd2,
        ):
            lhsT = const.tile([K_AUG, N], bf16)
            rhsT = const.tile([K_AUG, M], bf16)
            nc.sync.dma_start(out=lhsT, in_=lhs_d.ap())
            nc.sync.dma_start(out=rhsT, in_=rhs_d.ap())

            ident = const.tile([P, P], bf16)
            make_identity(nc, ident)

            acc = const.tile([P, M], bf16)      # running col-min
            xy = const.tile([P, 2 * TN], f32)   # [:, :TN] row mins, [:, TN:] col mins
            dist = const.tile([P, 2 * TN], f32)
            sums = const.tile([P, 1], f32)
            junk = scratch.tile([P, M], bf16)

            # preload the sqrt activation table during the ramp so the
            # ~2.7us ACT_TABLE_LOAD is not paid in the serial tail
            warm = const.tile([1, 1], f32)
            nc.vector.memset(warm, 1.0)
            nc.scalar.sqrt(warm, warm)

            loop_cm = tc.For_i(0, reps, 1) if reps > 1 else nullcontext()
            with loop_cm:
                for s in range(TN):
                    d2 = pd2.tile([P, M], f32, name="d2")
                    for j in range(M // NBLK):
                        nc.tensor.matmul(
                            d2[:, j * NBLK : (j + 1) * NBLK],
                            lhsT[:, s * P : (s + 1) * P],
                            rhsT[:, j * NBLK : (j + 1) * NBLK],
                            start=True,
                            stop=True,
                        )
                    bstrip = strips.tile([P, M], bf16, name="bstrip")
                    if s in FUSED_STRIPS:
                        # drain + row-min fused in one DVE op (1x, f32 PSUM)
                        nc.vector.tensor_scalar(
                            bstrip, d2, BIG, None, op0=Alu.min, op1=Alu.min,
                            accum_out=xy[:, s : s + 1],
                        )
                    else:
                        nc.scalar.copy(bstrip, d2)
                        nc.vector.tensor_scalar(
                            junk, bstrip, BIG, None, op0=Alu.min, op1=Alu.min,
                            accum_out=xy[:, s : s + 1],
                        )
                    # col-min accumulate (first strip seeds acc via 4x copy)
                    if s == 0:
                        nc.vector.tensor_copy(acc, bstrip)
                    else:
                        nc.vector.tensor_tensor(acc, acc, bstrip, op=Alu.min)

                # partition-min of acc via PE transposes + one multi-dim
                # reduce; accT borrows a rotating psum_d2 buffer (free after
                # the last strip's drain)
                accT = pd2.tile([P, TN, P], bf16, name="d2")
                for t in range(TN):
                    nc.tensor.transpose(
                        accT[:, t, :], acc[:, t * P : (t + 1) * P], ident
                    )
                nc.vector.tensor_reduce(
                    out=xy[:, TN : 2 * TN],
                    in_=accT,
                    axis=mybir.AxisListType.X,
                    op=Alu.min,
                )
                # d2 minima can round slightly negative; clamp before sqrt
                nc.vector.tensor_scalar_max(xy, xy, 0.0)
                nc.scalar.activation(
                    dist, xy, mybir.ActivationFunctionType.Sqrt, accum_out=sums
                )
                nc.sync.dma_start(out=out_d.ap(), in_=sums)

    nc.compile()
    return nc


def _split3(v):
    """3-way bf16 split: v ~= h + l + ll with ~2^-27 relative residual."""
    import ml_dtypes

    bf = ml_dtypes.bfloat16
    h = v.astype(bf)
    r = v - h.astype(np.float32)
    l = r.astype(bf)
    ll = (r - l.astype(np.float32)).astype(bf)
    return h, l, ll


def _prep_core(x, y):
    """Host-side per-core operand prep: O(N) layout, norms, bf16 splits.

    Summing lhsT[k]*rhs[k] over the 18 rows reconstructs
    |x|^2 + |y|^2 - 2 x.y with ~2^-27-scale absolute error (products of
    bf16 values are exact in the fp32 PSUM accumulator; only the
    representation residual and the dropped l*ll cross terms remain).
    Per coordinate (w = -2y): h*h', h*l', l*h', l*l', h*ll', ll*h'.
    Norms enter as 3-way splits against ones.
    """
    import ml_dtypes

    bf = ml_dtypes.bfloat16
    x = np.ascontiguousarray(x, dtype=np.float32)
    y = np.ascontiguousarray(y, dtype=np.float32)
    w = -2.0 * y
    nx = (x.astype(np.float64) ** 2).sum(axis=1).astype(np.float32)
    ny = (y.astype(np.float64) ** 2).sum(axis=1).astype(np.float32)

    lhs = np.empty((K_AUG, N), dtype=bf)
    rhs = np.empty((K_AUG, M), dtype=bf)
    k = 0
    for c in range(2):
        xh, xl, xll = _split3(x[:, c])
        wh, wl, wll = _split3(w[:, c])
        for a, b in ((xh, wh), (xh, wl), (xl, wh), (xl, wl), (xh, wll), (xll, wh)):
            lhs[k], rhs[k] = a, b
            k += 1
    one_n = np.ones(N, bf)
    one_m = np.ones(M, bf)
    for part in _split3(nx):
        lhs[k], rhs[k] = part, one_m
        k += 1
    for part in _split3(ny):
        lhs[k], rhs[k] = one_n, part
        k += 1
    assert k == K_AUG
    return {"lhs_aug": lhs, "rhs_aug": rhs}


def run(pds, pred_pds, reps=1, trace=None):
    global last_results
    from concourse import bass_utils

    pds = np.asarray(pds)
    pred_pds = np.asarray(pred_pds)
    assert pds.shape == (B, N, D) and pred_pds.shape == (B, M, D)

    if reps not in _nc_cache:
        _nc_cache[reps] = _build(reps)
    nc = _nc_cache[reps]

    in_maps = [_prep_core(pds[b], pred_pds[b]) for b in range(B)]
    last_results = bass_utils.run_bass_kernel_spmd(
        nc, in_maps, core_ids=list(range(B)),
        trace=TRACE if trace is None else trace,
    )
    vals = [
        float(last_results.results[b]["out"].sum()) / (2.0 * N) for b in range(B)
    ]
    return np.float32(np.mean(vals))


def kernel(pds, pred_pds):
    return run(pds, pred_pds, reps=1)
